# revision 29
# baseline (speedup 1.0000x reference)
"""Multi-LoRA routed adapter kernel for Trainium2 (8 NeuronCores).

Problem: out[b] = (x[b] @ B[aid[b]].T) @ A[aid[b]].T * (alpha/rank)
  x: [8, 1024, 2048] f32, A: [8, 2048, 16] f32, B: [8, 16, 2048] f32,
  adapter_ids: [8] i32, alpha/rank = 16/16 = 1.0.

Strategy: data-parallel over batch — sample b runs on core b. The
adapter gather (routing) is resolved host-side: each core receives only
its sample's selected A/B, pre-transposed so all device DMAs are
contiguous and the contraction dims land on SBUF partitions. All
tensors (including the OUTPUT) are fp16 on the wire: in+out HBM
traffic is ~4.8 MB + 4 MB per core (~24 us at the ~360 GB/s per-core
HBM limit). The host casts y back to f32 (measured end-to-end rel err
~6e-4, tolerance 2e-2).

Per-core device kernel, 4 pieces of 256 tokens:
  mm1 (col-tiled): the PE array is split into 4 column strips via
    tile_position=(0, 32j); strip j holds BT for k-tile group j and the
    4 strips stream their x chunks CONCURRENTLY, so mm1 costs ~1/4 of
    its serial form. Strip j writes Bx to PSUM partitions 32j..32j+15;
    hole partitions are pre-zeroed once and never written.
  mm2: lhsT = the full [128, 128-token] Bx slab (zero holes), rhs =
    AT128[p] = A^T[p mod 16] replicated host-side — the zero rows of
    lhsT null out the replicated junk, giving a full-K=128 matmul with
    the same N=512 stream count as the K=16 form.

Perf notes (measured):
  - single SP HWDGE ring for x + stores (loads first -> strict
    priority); AT128 rides the separate GpSimd SWDGE queue;
  - x pieces are loaded as two 512 KB half-K chunks so mm1 starts on
    the first chunk; slab-granular 512 KB stores track the PE;
  - the HAM clock gate holds the PE at 1.2 GHz until it has been busy
    one full ~3.4 us activity window (a ~0.5 us gap resets it): a
    gapless dummy accumulation stream on junk SBUF warms the PE before
    piece 0 lands;
  - PSUM has ONE VectorE read port (~1.2 ns/elem/partition): the
    PSUM->SBUF o-drain is split between VectorE and ScalarE on
    disjoint banks; the bx drain rides on ScalarE.
"""

import os

import numpy as np

import concourse.bass as bass
import concourse.mybir as mybir
import concourse.tile as tile
from concourse import bacc
from concourse.bass_utils import run_bass_kernel_spmd

# Problem constants (hardcoded per spec).
N_CORES = 8
BATCH = 8
N_TOK = 1024
D_IN = 2048
D_OUT = 2048
RANK = 16
SCALING = 16.0 / 16.0  # alpha / rank

P = 128
K_TILES = D_IN // P  # 16
KH = K_TILES // 2  # 8 k-tiles per load chunk
KG = 4  # k-tiles per PE column strip (4 strips)
PIECE = 256  # tokens per piece
N_PIECES = N_TOK // PIECE  # 4
SLABS = PIECE // P  # 2
O_CHUNK = 512  # one fp32 PSUM bank per matmul
N_WARM = 30

F32 = mybir.dt.float32
F16 = mybir.dt.float16

_last_results = None  # stashed BassKernelResults for test harness introspection
_nc_cache = None  # compiled Bass module, reused across kernel() calls


def _build_nc() -> bass.Bass:
    nc = bacc.Bacc(None, enable_asserts=False, enable_partition_id=False)
    # xp[pc, h, p, (kt-within-half)*PIECE + j] = x[b][pc*PIECE + j,
    # (h*KH + kt)*128 + p] — each (pc, h, p) row is 4 KB contiguous;
    # 512 KB per chunk, 2 chunks per piece.
    xp = nc.dram_tensor(
        "xp", [N_PIECES, 2, P, KH * PIECE], F16, kind="ExternalInput"
    )
    BTp = nc.dram_tensor("BTp", [P, K_TILES * RANK], F16, kind="ExternalInput")
    AT = nc.dram_tensor("AT", [RANK, D_OUT], F16, kind="ExternalInput")
    # Replication selector: E16[r, p] = (p % 16 == r). AT128 = E16^T @ AT.
    E16 = nc.dram_tensor("E16", [RANK, P], F16, kind="ExternalInput")
    y = nc.dram_tensor("y", [N_TOK, D_OUT], F16, kind="ExternalOutput")

    with tile.TileContext(nc) as tc:
        with (
            tc.tile_pool(name="const", bufs=1) as cpool,
            tc.tile_pool(name="xin", bufs=2 * N_PIECES) as xpool,
            tc.tile_pool(name="bx", bufs=2) as bxpool,
            tc.tile_pool(name="outb", bufs=4) as opool,
            tc.tile_pool(name="psbx", bufs=2, space="PSUM") as psbx,
            tc.tile_pool(name="pso", bufs=3, space="PSUM") as pso,
        ):
            # SP ring order: BT (tiny, feeds mm1), then the 8 x chunks,
            # then stores. AT128 goes on the GpSimd SWDGE queue so it
            # never delays the x stream.
            bt_sb = cpool.tile([P, K_TILES, RANK], F16)
            nc.sync.dma_start(
                bt_sb[:], BTp.rearrange("p (kt r) -> p kt r", r=RANK)
            )
            # AT/E16 (68 KB) ride the GpSimd SWDGE queue: the SP ring
            # carries nothing but the x stream ahead of the stores.
            at16_sb = cpool.tile([RANK, D_OUT], F16)
            nc.gpsimd.dma_start(at16_sb[:], AT[:, :])
            e16_sb = cpool.tile([RANK, P], F16)
            nc.gpsimd.dma_start(e16_sb[:], E16[:, :])
            x_sbs = []
            for pc in range(N_PIECES):
                halves = []
                for h in range(2):
                    x_sb = xpool.tile([P, KH, PIECE], F16, tag="x")
                    nc.sync.dma_start(
                        x_sb[:],
                        xp[pc, h].rearrange("p (kt n) -> p kt n", n=PIECE),
                    )
                    halves.append(x_sb)
                x_sbs.append(halves)

            # Pre-zero both PSUM bx slots: mm1's column strips write only
            # partitions 32j..32j+15; the hole partitions must stay zero
            # (they feed mm2's lhsT, nulling the replicated AT128 rows).
            # Matmul start=True only clears has_written bits, not data, so
            # one memset per slot lasts the whole kernel.
            # PE warm-up on uninitialized SBUF junk (no load dependency).
            # One gapless accumulation stream — any ~0.5 us PE gap resets
            # the HAM activity window and the PE stays at 1.2 GHz. The
            # junk memset is DVE's first instruction so warm-up starts
            # ~7.3 us, right after the engine barriers.
            junk = cpool.tile([P, P], F16)
            nc.vector.memset(junk[:], 1.0)

            zs = []
            for _ in range(2):
                z = psbx.tile([P, PIECE], F32, tag="ps_bx")
                nc.vector.memset(z[:], 0.0)
                zs.append(z)

            ps_w = psbx.tile([P, PIECE], F32, tag="ps_bx")
            for w in range(N_WARM):
                nc.tensor.matmul(
                    ps_w[:RANK, :P],
                    junk[:, :RANK],
                    junk[:],
                    start=(w == 0),
                    stop=(w == N_WARM - 1),
                )

            # Build AT128[p] = A^T[p % 16] on-device: 64 KB + 4 KB on the
            # wire instead of a 512 KB replicated const whose transfer
            # would contend with the critical early x chunks. The four
            # matmuls double as extra PE warm-up.
            at_sb = cpool.tile([P, D_OUT], F16)
            for half in range(2):
                ps_r = pso.tile([P, 2, O_CHUNK], F32, tag="ps_o")
                for q in range(2):
                    oc = 2 * half + q
                    nc.tensor.matmul(
                        ps_r[:, q, :],
                        e16_sb[:],
                        at16_sb[:, oc * O_CHUNK : (oc + 1) * O_CHUNK],
                        start=True,
                        stop=True,
                    )
                nc.vector.tensor_copy(
                    at_sb[:, 2 * half * O_CHUNK : 2 * (half + 1) * O_CHUNK],
                    ps_r[:, :, :],
                )

            def filler(n):
                # Col-tiled junk matmuls bridging load-receipt stalls: the
                # HAM activity window resets on any ~0.5 us PE gap, and a
                # reset mid-kernel re-throttles the PE to 1.2 GHz. These
                # occupy psbx slot A (parity: every mm1 lands on slot B).
                ps_f = psbx.tile([P, PIECE], F32, tag="ps_bx")
                for w in range(n):
                    nc.tensor.matmul(
                        ps_f[:RANK, :P],
                        junk[:, :RANK],
                        junk[:],
                        start=(w == 0),
                        stop=(w == n - 1),
                        tile_position=(0, 0),
                        skip_group_check=True,
                    )

            def mm1(pc):
                # 4 concurrent column strips; strip j accumulates k-tile
                # group j (kt = 4j..4j+3) into PSUM partitions 32j..32j+15.
                # The has_written clear of start=True is region-scoped
                # (measured), so each strip opens its own accumulation
                # group with k==0.
                ps_bx = psbx.tile([P, PIECE], F32, tag="ps_bx")
                for h in range(2):  # load-half: strips 2h, 2h+1
                    for k in range(KG):
                        for j in (2 * h, 2 * h + 1):
                            kt = j * KG + k
                            kh = kt - h * KH
                            nc.tensor.matmul(
                                ps_bx[32 * j : 32 * j + RANK, :],
                                bt_sb[:, kt, :],
                                x_sbs[pc][h][:, kh, :],
                                start=(k == 0),
                                stop=(k == KG - 1),
                                tile_position=(0, 32 * j),
                                skip_group_check=True,
                            )
                bx_sb = bxpool.tile([P, PIECE], F16)
                # bx drain on ACT: DVE's queue is busy with o-drains.
                nc.scalar.copy(bx_sb[:], ps_bx[:])
                return bx_sb

            def mm2(bx_sb, pc):
                for s in range(SLABS):
                    o_sb = opool.tile([P, D_OUT], F16, tag="o")
                    for half in range(2):
                        ps_o = pso.tile([P, 2, O_CHUNK], F32)
                        for q in range(2):
                            oc = 2 * half + q
                            nc.tensor.matmul(
                                ps_o[:, q, :],
                                bx_sb[:, s * P : (s + 1) * P],
                                at_sb[:, oc * O_CHUNK : (oc + 1) * O_CHUNK],
                                start=True,
                                stop=True,
                            )
                        # Drain split: DVE half 0, ACT half 1 (disjoint
                        # PSUM banks, runs in parallel on TRN2).
                        dst = o_sb[:, 2 * half * O_CHUNK : 2 * (half + 1) * O_CHUNK]
                        if half == 0:
                            nc.vector.tensor_copy(dst, ps_o[:, :, :])
                        else:
                            nc.scalar.copy(dst, ps_o[:, :, :])
                    # Slab-granular store (512 KB): the out stream is
                    # readiness-gated, not ring-gated.
                    row0 = pc * PIECE + s * P
                    nc.sync.dma_start(y[row0 : row0 + P, :], o_sb[:])

            # Software pipeline: mm1 one piece ahead of mm2. The PE order
            # (warmup -> AT128 replication -> mm1/mm2 stream) is gapless
            # through the HAM activity window, so the whole kernel runs at
            # 2.4 GHz.
            FILL = {1: 14, 2: 8, 3: 6}
            bx_prev = mm1(0)
            for pc in range(N_PIECES):
                if pc + 1 < N_PIECES:
                    filler(FILL[pc + 1])
                    bx_next = mm1(pc + 1)
                else:
                    bx_next = None
                mm2(bx_prev, pc)
                bx_prev = bx_next
    nc.compile()
    return nc


def kernel(x, A, B, adapter_ids):
    global _last_results
    x = np.asarray(x, dtype=np.float32)
    A = np.asarray(A, dtype=np.float32)
    B = np.asarray(B, dtype=np.float32)
    adapter_ids = np.asarray(adapter_ids)

    assert x.shape == (BATCH, N_TOK, D_IN)

    in_maps = []
    for b in range(BATCH):
        aid = int(adapter_ids[b])
        # Fold the LoRA scaling into A (scaling is 1.0 here, exact).
        At = np.ascontiguousarray(
            (A[aid].T * np.float32(SCALING)).astype(np.float16)
        )
        # Pack B^T to [p, kt*r]: BTp[p, kt*16+r] = B^T[kt*128+p, r]
        BTp = np.ascontiguousarray(
            B[aid].T.reshape(K_TILES, P, RANK)
            .transpose(1, 0, 2)
            .reshape(P, K_TILES * RANK)
            .astype(np.float16)
        )
        # [pc, j, h, kt, p] -> [pc, h, p, kt, j]: 4 KB rows per chunk.
        xp = np.ascontiguousarray(
            x[b]
            .reshape(N_PIECES, PIECE, 2, KH, P)
            .transpose(0, 2, 4, 3, 1)
            .reshape(N_PIECES, 2, P, KH * PIECE)
            .astype(np.float16)
        )
        e16 = np.zeros((RANK, P), np.float16)
        e16[np.arange(P) % RANK, np.arange(P)] = 1.0
        in_maps.append({"xp": xp, "BTp": BTp, "AT": At, "E16": e16})

    global _nc_cache
    if _nc_cache is None:
        _nc_cache = _build_nc()
    nc = _nc_cache
    trace = bool(int(os.environ.get("KERNEL_BASS_TRACE", "0")))
    res = run_bass_kernel_spmd(
        nc, in_maps, core_ids=list(range(N_CORES)), trace=trace
    )
    _last_results = res

    out = np.empty((BATCH, N_TOK, D_OUT), dtype=np.float32)
    for b in range(BATCH):
        out[b] = res.results[b]["y"].astype(np.float32)
    return out


# revision 30
# speedup vs baseline: 1.0638x; 1.0638x over previous
"""Multi-LoRA routed adapter kernel for Trainium2 (8 NeuronCores).

Problem: out[b] = (x[b] @ B[aid[b]].T) @ A[aid[b]].T * (alpha/rank)
  x: [8, 1024, 2048] f32, A: [8, 2048, 16] f32, B: [8, 16, 2048] f32,
  adapter_ids: [8] i32, alpha/rank = 16/16 = 1.0.

Strategy: data-parallel over batch — sample b runs on core b. The
adapter gather (routing) is resolved host-side: each core receives only
its sample's selected A/B, pre-transposed so all device DMAs are
contiguous and the contraction dims land on SBUF partitions. All
tensors (including the OUTPUT) are fp16 on the wire: in+out HBM
traffic is ~4.8 MB + 4 MB per core (~24 us at the ~360 GB/s per-core
HBM limit). The host casts y back to f32 (measured end-to-end rel err
~6e-4, tolerance 2e-2).

Per-core device kernel, 4 pieces of 256 tokens:
  mm1 (col-tiled): the PE array is split into 4 column strips via
    tile_position=(0, 32j); strip j holds BT for k-tile group j and the
    4 strips stream their x chunks CONCURRENTLY, so mm1 costs ~1/4 of
    its serial form. Strip j writes Bx to PSUM partitions 32j..32j+15;
    hole partitions are pre-zeroed once and never written.
  mm2: lhsT = the full [128, 128-token] Bx slab (zero holes), rhs =
    AT128[p] = A^T[p mod 16] replicated host-side — the zero rows of
    lhsT null out the replicated junk, giving a full-K=128 matmul with
    the same N=512 stream count as the K=16 form.

Perf notes (measured):
  - single SP HWDGE ring for x + stores (loads first -> strict
    priority); AT128 rides the separate GpSimd SWDGE queue;
  - x pieces are loaded as two 512 KB half-K chunks so mm1 starts on
    the first chunk; slab-granular 512 KB stores track the PE;
  - the HAM clock gate holds the PE at 1.2 GHz until it has been busy
    one full ~3.4 us activity window (a ~0.5 us gap resets it): a
    gapless dummy accumulation stream on junk SBUF warms the PE before
    piece 0 lands;
  - PSUM has ONE VectorE read port (~1.2 ns/elem/partition): the
    PSUM->SBUF o-drain is split between VectorE and ScalarE on
    disjoint banks; the bx drain rides on ScalarE.
"""

import os

import numpy as np

import concourse.bass as bass
import concourse.mybir as mybir
import concourse.tile as tile
from concourse import bacc
from concourse.bass_utils import run_bass_kernel_spmd

# Problem constants (hardcoded per spec).
N_CORES = 8
BATCH = 8
N_TOK = 1024
D_IN = 2048
D_OUT = 2048
RANK = 16
SCALING = 16.0 / 16.0  # alpha / rank

P = 128
K_TILES = D_IN // P  # 16
KH = K_TILES // 2  # 8 k-tiles per load chunk
KG = 4  # k-tiles per PE column strip (4 strips)
PIECE = 256  # tokens per piece
N_PIECES = N_TOK // PIECE  # 4
SLABS = PIECE // P  # 2
O_CHUNK = 512  # one fp32 PSUM bank per matmul
N_WARM = 30

F32 = mybir.dt.float32
F16 = mybir.dt.float16

_last_results = None  # stashed BassKernelResults for test harness introspection
_nc_cache = None  # compiled Bass module, reused across kernel() calls


def _build_nc() -> bass.Bass:
    nc = bacc.Bacc(None, enable_asserts=False, enable_partition_id=False)
    # xp[pc, h, p, (kt-within-half)*PIECE + j] = x[b][pc*PIECE + j,
    # (h*KH + kt)*128 + p] — each (pc, h, p) row is 4 KB contiguous;
    # 512 KB per chunk, 2 chunks per piece.
    xp = nc.dram_tensor(
        "xp", [N_PIECES, 2, P, KH * PIECE], F16, kind="ExternalInput"
    )
    BTp = nc.dram_tensor("BTp", [P, K_TILES * RANK], F16, kind="ExternalInput")
    AT = nc.dram_tensor("AT", [RANK, D_OUT], F16, kind="ExternalInput")
    # Replication selector: E16[r, p] = (p % 16 == r). AT128 = E16^T @ AT.
    E16 = nc.dram_tensor("E16", [RANK, P], F16, kind="ExternalInput")
    y = nc.dram_tensor("y", [N_TOK, D_OUT], F16, kind="ExternalOutput")

    with tile.TileContext(nc) as tc:
        with (
            tc.tile_pool(name="const", bufs=1) as cpool,
            tc.tile_pool(name="xin", bufs=2 * N_PIECES) as xpool,
            tc.tile_pool(name="bx", bufs=2) as bxpool,
            tc.tile_pool(name="outb", bufs=4) as opool,
            tc.tile_pool(name="psbx", bufs=2, space="PSUM") as psbx,
            tc.tile_pool(name="pso", bufs=3, space="PSUM") as pso,
        ):
            # SP ring order: BT (tiny, feeds mm1), then the 8 x chunks,
            # then stores. AT128 goes on the GpSimd SWDGE queue so it
            # never delays the x stream.
            bt_sb = cpool.tile([P, K_TILES, RANK], F16)
            nc.sync.dma_start(
                bt_sb[:], BTp.rearrange("p (kt r) -> p kt r", r=RANK)
            )
            # AT/E16 (68 KB) ride the GpSimd SWDGE queue: the SP ring
            # carries nothing but the x stream ahead of the stores.
            at16_sb = cpool.tile([RANK, D_OUT], F16)
            nc.gpsimd.dma_start(at16_sb[:], AT[:, :])
            e16_sb = cpool.tile([RANK, P], F16)
            nc.gpsimd.dma_start(e16_sb[:], E16[:, :])
            x_sbs = []
            for pc in range(N_PIECES):
                halves = []
                for h in range(2):
                    x_sb = xpool.tile([P, KH, PIECE], F16, tag="x")
                    nc.sync.dma_start(
                        x_sb[:],
                        xp[pc, h].rearrange("p (kt n) -> p kt n", n=PIECE),
                    )
                    halves.append(x_sb)
                x_sbs.append(halves)

            # Pre-zero both PSUM bx slots: mm1's column strips write only
            # partitions 32j..32j+15; the hole partitions must stay zero
            # (they feed mm2's lhsT, nulling the replicated AT128 rows).
            # Matmul start=True only clears has_written bits, not data, so
            # one memset per slot lasts the whole kernel.
            # PE warm-up on uninitialized SBUF junk (no load dependency).
            # One gapless accumulation stream — any ~0.5 us PE gap resets
            # the HAM activity window and the PE stays at 1.2 GHz. The
            # junk memset is DVE's first instruction so warm-up starts
            # ~7.3 us, right after the engine barriers.
            junk = cpool.tile([P, P], F16)
            nc.vector.memset(junk[:], 1.0)

            zs = []
            for _ in range(2):
                z = psbx.tile([P, PIECE], F32, tag="ps_bx")
                nc.vector.memset(z[:], 0.0)
                zs.append(z)

            ps_w = psbx.tile([P, PIECE], F32, tag="ps_bx")
            for w in range(N_WARM):
                nc.tensor.matmul(
                    ps_w[:RANK, :P],
                    junk[:, :RANK],
                    junk[:],
                    start=(w == 0),
                    stop=(w == N_WARM - 1),
                )

            # Build AT128[p] = A^T[p % 16] on-device: 64 KB + 4 KB on the
            # wire instead of a 512 KB replicated const whose transfer
            # would contend with the critical early x chunks. The four
            # matmuls double as extra PE warm-up.
            at_sb = cpool.tile([P, D_OUT], F16)
            for half in range(2):
                ps_r = pso.tile([P, 2, O_CHUNK], F32, tag="ps_o")
                for q in range(2):
                    oc = 2 * half + q
                    nc.tensor.matmul(
                        ps_r[:, q, :],
                        e16_sb[:],
                        at16_sb[:, oc * O_CHUNK : (oc + 1) * O_CHUNK],
                        start=True,
                        stop=True,
                    )
                nc.vector.tensor_copy(
                    at_sb[:, 2 * half * O_CHUNK : 2 * (half + 1) * O_CHUNK],
                    ps_r[:, :, :],
                )

            def filler(n):
                # Col-tiled junk matmuls bridging load-receipt stalls: the
                # HAM activity window resets on any ~0.5 us PE gap, and a
                # reset mid-kernel re-throttles the PE to 1.2 GHz. These
                # occupy psbx slot A (parity: every mm1 lands on slot B).
                ps_f = psbx.tile([P, PIECE], F32, tag="ps_bx")
                for w in range(n):
                    nc.tensor.matmul(
                        ps_f[:RANK, :P],
                        junk[:, :RANK],
                        junk[:],
                        start=(w == 0),
                        stop=(w == n - 1),
                        tile_position=(0, 0),
                        skip_group_check=True,
                    )

            def mm1(pc):
                # 4 concurrent column strips; strip j accumulates k-tile
                # group j (kt = 4j..4j+3) into PSUM partitions 32j..32j+15.
                # The has_written clear of start=True is region-scoped
                # (measured), so each strip opens its own accumulation
                # group with k==0.
                ps_bx = psbx.tile([P, PIECE], F32, tag="ps_bx")
                for h in range(2):  # load-half: strips 2h, 2h+1
                    for k in range(KG):
                        for j in (2 * h, 2 * h + 1):
                            kt = j * KG + k
                            kh = kt - h * KH
                            nc.tensor.matmul(
                                ps_bx[32 * j : 32 * j + RANK, :],
                                bt_sb[:, kt, :],
                                x_sbs[pc][h][:, kh, :],
                                start=(k == 0),
                                stop=(k == KG - 1),
                                tile_position=(0, 32 * j),
                                skip_group_check=True,
                            )
                bx_sb = bxpool.tile([P, PIECE], F16)
                # bx drain on ACT: DVE's queue is busy with o-drains.
                nc.scalar.copy(bx_sb[:], ps_bx[:])
                return bx_sb

            def mm2(bx_sb, pc):
                for s in range(SLABS):
                    o_sb = opool.tile([P, D_OUT], F16, tag="o")
                    for half in range(2):
                        ps_o = pso.tile([P, 2, O_CHUNK], F32)
                        for q in range(2):
                            oc = 2 * half + q
                            nc.tensor.matmul(
                                ps_o[:, q, :],
                                bx_sb[:, s * P : (s + 1) * P],
                                at_sb[:, oc * O_CHUNK : (oc + 1) * O_CHUNK],
                                start=True,
                                stop=True,
                            )
                        # Drain split: DVE half 0, ACT half 1 (disjoint
                        # PSUM banks, runs in parallel on TRN2).
                        dst = o_sb[:, 2 * half * O_CHUNK : 2 * (half + 1) * O_CHUNK]
                        if half == 0:
                            nc.vector.tensor_copy(dst, ps_o[:, :, :])
                        else:
                            nc.scalar.copy(dst, ps_o[:, :, :])
                    # Slab-granular store (512 KB): the out stream is
                    # readiness-gated, not ring-gated.
                    row0 = pc * PIECE + s * P
                    nc.sync.dma_start(y[row0 : row0 + P, :], o_sb[:])

            # Software pipeline: mm1 one piece ahead of mm2. The PE order
            # (warmup -> AT128 replication -> mm1/mm2 stream) is gapless
            # through the HAM activity window, so the whole kernel runs at
            # 2.4 GHz.
            FILL = {1: 40, 2: 24, 3: 14}
            bx_prev = mm1(0)
            for pc in range(N_PIECES):
                if pc + 1 < N_PIECES:
                    filler(FILL[pc + 1])
                    bx_next = mm1(pc + 1)
                else:
                    bx_next = None
                mm2(bx_prev, pc)
                bx_prev = bx_next
    nc.compile()
    return nc


def kernel(x, A, B, adapter_ids):
    global _last_results
    x = np.asarray(x, dtype=np.float32)
    A = np.asarray(A, dtype=np.float32)
    B = np.asarray(B, dtype=np.float32)
    adapter_ids = np.asarray(adapter_ids)

    assert x.shape == (BATCH, N_TOK, D_IN)

    in_maps = []
    for b in range(BATCH):
        aid = int(adapter_ids[b])
        # Fold the LoRA scaling into A (scaling is 1.0 here, exact).
        At = np.ascontiguousarray(
            (A[aid].T * np.float32(SCALING)).astype(np.float16)
        )
        # Pack B^T to [p, kt*r]: BTp[p, kt*16+r] = B^T[kt*128+p, r]
        BTp = np.ascontiguousarray(
            B[aid].T.reshape(K_TILES, P, RANK)
            .transpose(1, 0, 2)
            .reshape(P, K_TILES * RANK)
            .astype(np.float16)
        )
        # [pc, j, h, kt, p] -> [pc, h, p, kt, j]: 4 KB rows per chunk.
        xp = np.ascontiguousarray(
            x[b]
            .reshape(N_PIECES, PIECE, 2, KH, P)
            .transpose(0, 2, 4, 3, 1)
            .reshape(N_PIECES, 2, P, KH * PIECE)
            .astype(np.float16)
        )
        e16 = np.zeros((RANK, P), np.float16)
        e16[np.arange(P) % RANK, np.arange(P)] = 1.0
        in_maps.append({"xp": xp, "BTp": BTp, "AT": At, "E16": e16})

    global _nc_cache
    if _nc_cache is None:
        _nc_cache = _build_nc()
    nc = _nc_cache
    trace = bool(int(os.environ.get("KERNEL_BASS_TRACE", "0")))
    res = run_bass_kernel_spmd(
        nc, in_maps, core_ids=list(range(N_CORES)), trace=trace
    )
    _last_results = res

    out = np.empty((BATCH, N_TOK, D_OUT), dtype=np.float32)
    for b in range(BATCH):
        out[b] = res.results[b]["y"].astype(np.float32)
    return out


# revision 34
# speedup vs baseline: 1.0964x; 1.0306x over previous
"""Multi-LoRA routed adapter kernel for Trainium2 (8 NeuronCores).

Problem: out[b] = (x[b] @ B[aid[b]].T) @ A[aid[b]].T * (alpha/rank)
  x: [8, 1024, 2048] f32, A: [8, 2048, 16] f32, B: [8, 16, 2048] f32,
  adapter_ids: [8] i32, alpha/rank = 16/16 = 1.0.

Strategy: data-parallel over batch — sample b runs on core b. The
adapter gather (routing) is resolved host-side: each core receives only
its sample's selected A/B, pre-transposed so all device DMAs are
contiguous and the contraction dims land on SBUF partitions. All
tensors (including the OUTPUT) are fp16 on the wire: in+out HBM
traffic is ~4.8 MB + 4 MB per core (~24 us at the ~360 GB/s per-core
HBM limit). The host casts y back to f32 (measured end-to-end rel err
~6e-4, tolerance 2e-2).

Per-core device kernel, 4 pieces of 256 tokens:
  mm1 (col-tiled): the PE array is split into 4 column strips via
    tile_position=(0, 32j); strip j holds BT for k-tile group j and the
    4 strips stream their x chunks CONCURRENTLY, so mm1 costs ~1/4 of
    its serial form. Strip j writes Bx to PSUM partitions 32j..32j+15;
    hole partitions are pre-zeroed once and never written.
  mm2: lhsT = the full [128, 128-token] Bx slab (zero holes), rhs =
    AT128[p] = A^T[p mod 16] replicated host-side — the zero rows of
    lhsT null out the replicated junk, giving a full-K=128 matmul with
    the same N=512 stream count as the K=16 form.

Perf notes (measured):
  - single SP HWDGE ring for x + stores (loads first -> strict
    priority); AT128 rides the separate GpSimd SWDGE queue;
  - x pieces are loaded as two 512 KB half-K chunks so mm1 starts on
    the first chunk; slab-granular 512 KB stores track the PE;
  - the HAM clock gate holds the PE at 1.2 GHz until it has been busy
    one full ~3.4 us activity window (a ~0.5 us gap resets it): a
    gapless dummy accumulation stream on junk SBUF warms the PE before
    piece 0 lands;
  - PSUM has ONE VectorE read port (~1.2 ns/elem/partition): the
    PSUM->SBUF o-drain is split between VectorE and ScalarE on
    disjoint banks; the bx drain rides on ScalarE.
"""

import os

import numpy as np

import concourse.bass as bass
import concourse.mybir as mybir
import concourse.tile as tile
from concourse import bacc
from concourse.bass_utils import run_bass_kernel_spmd

# Problem constants (hardcoded per spec).
N_CORES = 8
BATCH = 8
N_TOK = 1024
D_IN = 2048
D_OUT = 2048
RANK = 16
SCALING = 16.0 / 16.0  # alpha / rank

P = 128
K_TILES = D_IN // P  # 16
KH = K_TILES // 2  # 8 k-tiles per load chunk
KG = 4  # k-tiles per PE column strip (4 strips)
PIECE = 256  # tokens per piece
N_PIECES = N_TOK // PIECE  # 4
SLABS = PIECE // P  # 2
O_CHUNK = 512  # one fp32 PSUM bank per matmul
N_WARM = 36

F32 = mybir.dt.float32
F16 = mybir.dt.float16

_last_results = None  # stashed BassKernelResults for test harness introspection
_nc_cache = None  # compiled Bass module, reused across kernel() calls


def _build_nc() -> bass.Bass:
    nc = bacc.Bacc(None, enable_asserts=False, enable_partition_id=False)
    # xp[pc, h, p, (kt-within-half)*PIECE + j] = x[b][pc*PIECE + j,
    # (h*KH + kt)*128 + p] — each (pc, h, p) row is 4 KB contiguous;
    # 512 KB per chunk, 2 chunks per piece.
    xp = nc.dram_tensor(
        "xp", [N_PIECES, 2, P, KH * PIECE], F16, kind="ExternalInput"
    )
    BTp = nc.dram_tensor("BTp", [P, K_TILES * RANK], F16, kind="ExternalInput")
    AT = nc.dram_tensor("AT", [RANK, D_OUT], F16, kind="ExternalInput")
    # Replication selector: E16[r, p] = (p % 16 == r). AT128 = E16^T @ AT.
    E16 = nc.dram_tensor("E16", [RANK, P], F16, kind="ExternalInput")
    y = nc.dram_tensor("y", [N_TOK, D_OUT], F16, kind="ExternalOutput")

    with tile.TileContext(nc) as tc:
        with (
            tc.tile_pool(name="const", bufs=1) as cpool,
            tc.tile_pool(name="xin", bufs=2 * N_PIECES) as xpool,
            tc.tile_pool(name="bx", bufs=2) as bxpool,
            tc.tile_pool(name="outb", bufs=4) as opool,
            tc.tile_pool(name="psbx", bufs=2, space="PSUM") as psbx,
            tc.tile_pool(name="pso", bufs=3, space="PSUM") as pso,
        ):
            # SP ring order: BT (tiny, feeds mm1), then the 8 x chunks,
            # then stores. AT128 goes on the GpSimd SWDGE queue so it
            # never delays the x stream.
            bt_sb = cpool.tile([P, K_TILES, RANK], F16)
            nc.sync.dma_start(
                bt_sb[:], BTp.rearrange("p (kt r) -> p kt r", r=RANK)
            )
            # AT/E16 (68 KB) ride the GpSimd SWDGE queue: the SP ring
            # carries nothing but the x stream ahead of the stores.
            at16_sb = cpool.tile([RANK, D_OUT], F16)
            nc.gpsimd.dma_start(at16_sb[:], AT[:, :])
            e16_sb = cpool.tile([RANK, P], F16)
            nc.gpsimd.dma_start(e16_sb[:], E16[:, :])
            x_sbs = []
            for pc in range(N_PIECES):
                halves = []
                for h in range(2):
                    x_sb = xpool.tile([P, KH, PIECE], F16, tag="x")
                    nc.sync.dma_start(
                        x_sb[:],
                        xp[pc, h].rearrange("p (kt n) -> p kt n", n=PIECE),
                    )
                    halves.append(x_sb)
                x_sbs.append(halves)

            # Pre-zero both PSUM bx slots: mm1's column strips write only
            # partitions 32j..32j+15; the hole partitions must stay zero
            # (they feed mm2's lhsT, nulling the replicated AT128 rows).
            # Matmul start=True only clears has_written bits, not data, so
            # one memset per slot lasts the whole kernel.
            # PE warm-up on uninitialized SBUF junk (no load dependency).
            # One gapless accumulation stream — any ~0.5 us PE gap resets
            # the HAM activity window and the PE stays at 1.2 GHz. The
            # junk memset is DVE's first instruction so warm-up starts
            # ~7.3 us, right after the engine barriers.
            junk = cpool.tile([P, P], F16)
            nc.vector.memset(junk[:], 1.0)

            zs = []
            for _ in range(2):
                z = psbx.tile([P, PIECE], F32, tag="ps_bx")
                nc.vector.memset(z[:], 0.0)
                zs.append(z)

            ps_w = psbx.tile([P, PIECE], F32, tag="ps_bx")
            for w in range(N_WARM):
                nc.tensor.matmul(
                    ps_w[:RANK, :P],
                    junk[:, :RANK],
                    junk[:],
                    start=(w == 0),
                    stop=(w == N_WARM - 1),
                )

            # Build AT128[p] = A^T[p % 16] on-device: 64 KB + 4 KB on the
            # wire instead of a 512 KB replicated const whose transfer
            # would contend with the critical early x chunks. The four
            # matmuls double as extra PE warm-up.
            at_sb = cpool.tile([P, D_OUT], F16)
            for half in range(2):
                ps_r = pso.tile([P, 2, O_CHUNK], F32, tag="ps_o")
                for q in range(2):
                    oc = 2 * half + q
                    nc.tensor.matmul(
                        ps_r[:, q, :],
                        e16_sb[:],
                        at16_sb[:, oc * O_CHUNK : (oc + 1) * O_CHUNK],
                        start=True,
                        stop=True,
                    )
                nc.vector.tensor_copy(
                    at_sb[:, 2 * half * O_CHUNK : 2 * (half + 1) * O_CHUNK],
                    ps_r[:, :, :],
                )

            def mm1(pc):
                # 4 concurrent column strips; strip j accumulates k-tile
                # group j (kt = 4j..4j+3) into PSUM partitions 32j..32j+15.
                # The has_written clear of start=True is region-scoped
                # (measured), so each strip opens its own accumulation
                # group with k==0.
                ps_bx = psbx.tile([P, PIECE], F32, tag="ps_bx")
                for h in range(2):  # load-half: strips 2h, 2h+1
                    for k in range(KG):
                        for j in (2 * h, 2 * h + 1):
                            kt = j * KG + k
                            kh = kt - h * KH
                            nc.tensor.matmul(
                                ps_bx[32 * j : 32 * j + RANK, :],
                                bt_sb[:, kt, :],
                                x_sbs[pc][h][:, kh, :],
                                start=(k == 0),
                                stop=(k == KG - 1),
                                tile_position=(0, 32 * j),
                                skip_group_check=True,
                            )
                bx_sb = bxpool.tile([P, PIECE], F16)
                # bx drain on ACT: DVE's queue is busy with o-drains.
                nc.scalar.copy(bx_sb[:], ps_bx[:])
                return bx_sb

            def mm2(bx_sb, pc):
                last = pc == N_PIECES - 1
                for s in range(SLABS):
                    o_sb = opool.tile([P, D_OUT], F16, tag="o")
                    for half in range(2):
                        ps_o = pso.tile([P, 2, O_CHUNK], F32)
                        for q in range(2):
                            oc = 2 * half + q
                            nc.tensor.matmul(
                                ps_o[:, q, :],
                                bx_sb[:, s * P : (s + 1) * P],
                                at_sb[:, oc * O_CHUNK : (oc + 1) * O_CHUNK],
                                start=True,
                                stop=True,
                            )
                        # Drain split: DVE half 0, ACT half 1 (disjoint
                        # PSUM banks, runs in parallel on TRN2).
                        dst = o_sb[:, 2 * half * O_CHUNK : 2 * (half + 1) * O_CHUNK]
                        if half == 0:
                            nc.vector.tensor_copy(dst, ps_o[:, :, :])
                        else:
                            nc.scalar.copy(dst, ps_o[:, :, :])
                        if last and s == SLABS - 1:
                            # Final slab: store each half as soon as its
                            # drain engine finishes — shaves the kernel
                            # tail by ~half a drain.
                            row0 = pc * PIECE + s * P
                            nc.sync.dma_start(
                                y[
                                    row0 : row0 + P,
                                    2 * half * O_CHUNK : 2 * (half + 1) * O_CHUNK,
                                ],
                                dst,
                            )
                    if not (last and s == SLABS - 1):
                        # Slab-granular store (512 KB): the out stream is
                        # readiness-gated, not ring-gated.
                        row0 = pc * PIECE + s * P
                        nc.sync.dma_start(y[row0 : row0 + P, :], o_sb[:])

            # Software pipeline: mm1 one piece ahead of mm2. The PE order
            # (warmup -> AT128 replication -> mm1/mm2 stream) is gapless
            # through the HAM activity window, so the whole kernel runs at
            # 2.4 GHz.
            bx_prev = mm1(0)
            for pc in range(N_PIECES):
                bx_next = mm1(pc + 1) if pc + 1 < N_PIECES else None
                mm2(bx_prev, pc)
                bx_prev = bx_next
    nc.compile()
    return nc


def kernel(x, A, B, adapter_ids):
    global _last_results
    x = np.asarray(x, dtype=np.float32)
    A = np.asarray(A, dtype=np.float32)
    B = np.asarray(B, dtype=np.float32)
    adapter_ids = np.asarray(adapter_ids)

    assert x.shape == (BATCH, N_TOK, D_IN)

    in_maps = []
    for b in range(BATCH):
        aid = int(adapter_ids[b])
        # Fold the LoRA scaling into A (scaling is 1.0 here, exact).
        At = np.ascontiguousarray(
            (A[aid].T * np.float32(SCALING)).astype(np.float16)
        )
        # Pack B^T to [p, kt*r]: BTp[p, kt*16+r] = B^T[kt*128+p, r]
        BTp = np.ascontiguousarray(
            B[aid].T.reshape(K_TILES, P, RANK)
            .transpose(1, 0, 2)
            .reshape(P, K_TILES * RANK)
            .astype(np.float16)
        )
        # [pc, j, h, kt, p] -> [pc, h, p, kt, j]: 4 KB rows per chunk.
        xp = np.ascontiguousarray(
            x[b]
            .reshape(N_PIECES, PIECE, 2, KH, P)
            .transpose(0, 2, 4, 3, 1)
            .reshape(N_PIECES, 2, P, KH * PIECE)
            .astype(np.float16)
        )
        e16 = np.zeros((RANK, P), np.float16)
        e16[np.arange(P) % RANK, np.arange(P)] = 1.0
        in_maps.append({"xp": xp, "BTp": BTp, "AT": At, "E16": e16})

    global _nc_cache
    if _nc_cache is None:
        _nc_cache = _build_nc()
    nc = _nc_cache
    trace = bool(int(os.environ.get("KERNEL_BASS_TRACE", "0")))
    res = run_bass_kernel_spmd(
        nc, in_maps, core_ids=list(range(N_CORES)), trace=trace
    )
    _last_results = res

    out = np.empty((BATCH, N_TOK, D_OUT), dtype=np.float32)
    for b in range(BATCH):
        out[b] = res.results[b]["y"].astype(np.float32)
    return out
